# revision 21
# baseline (speedup 1.0000x reference)
"""Gemma3 sliding-window attention on 8 TRN2 NeuronCores via Bass/Tile.

Sharding: core c = b*4 + g  (b = batch, g = head-group):
  - q-heads {2g, 2g+1}, kv-head g, batch b  -> zero redundant projection work
  - column-shard wq/wk/wv, RoPE+RMSNorm local, blocked sliding-window
    attention, then a 4-rank bf16 AllGather of the attention outputs
    (concat over heads) per batch group, and a column-shard of wo.
All tensor-engine work in bf16 (f32 PSUM accumulation); softmax in f32->bf16.

Layout is fully transposed on-chip: Q^T/K^T are [head_dim, tokens] straight
out of the projection matmuls, V is [tokens, head_dim], scores are S^T
[keys, queries], attention output is O^T [head_dim, tokens] which directly
feeds the o-proj contraction. No transposes anywhere.

Pipelining: the o-proj for each query block is deferred by a full phase after
its AllGather, and the last two blocks of each rep drain during the NEXT
rep's projection phases, so the PE never waits on a collective. Softmax
denominators and RMS sums are computed with a ones[128,128] matmul so the
reciprocal runs as a full-width [128,512] DVE op (the [1,512] form is
lane-serial and ~8x slower).
"""
import sys
import numpy as np

if "/opt/trn_rl_repo" not in sys.path:
    sys.path.insert(0, "/opt/trn_rl_repo")

from contextlib import ExitStack

import ml_dtypes
import concourse.bass as bass
import concourse.mybir as mybir
import concourse.tile as tile
from concourse import bacc
from concourse import bass_utils

BF16 = mybir.dt.bfloat16
F32 = mybir.dt.float32
NPBF16 = ml_dtypes.bfloat16

B, S, HID = 2, 2048, 2560
NH, NKV, HD = 8, 4, 256
SCALE = 256.0 ** -0.5
WIN = 1024
NCORES = 8
KT = HID // 128          # 20 k-tiles over hidden dim
QB = 512                 # query block (free dim of score matmuls)
NQB = S // QB            # 4 query blocks
TM = S // 128            # 16 token tiles of 128
NHID_LOC = 640           # per-core slice of o-proj output columns

REPLICA_GROUPS = [[0, 1, 2, 3], [4, 5, 6, 7]]


def _emit(nc, reps=1, stub_collective=False):
    AF = mybir.ActivationFunctionType
    ALU = mybir.AluOpType

    xT = nc.dram_tensor("xT", [HID, S], BF16, kind="ExternalInput")
    wq = nc.dram_tensor("wq", [HID, 512], BF16, kind="ExternalInput")
    wk = nc.dram_tensor("wk", [HID, 256], BF16, kind="ExternalInput")
    wv = nc.dram_tensor("wv", [HID, 256], BF16, kind="ExternalInput")
    wo = nc.dram_tensor("wo", [NH * HD, NHID_LOC], BF16, kind="ExternalInput")
    cosT = nc.dram_tensor("cosT", [HD, S], BF16, kind="ExternalInput")
    rsinT = nc.dram_tensor("rsinT", [HD, S], BF16, kind="ExternalInput")
    qnw = nc.dram_tensor("qnw", [HD, 1], F32, kind="ExternalInput")
    knw = nc.dram_tensor("knw", [HD, 1], F32, kind="ExternalInput")
    maskb = nc.dram_tensor("maskb", [128, 1152], BF16, kind="ExternalInput")
    # out is transposed ([o-cols, tokens]): the o-proj runs with wo column
    # chunks stationary and the gathered activations moving, so every o-proj
    # matmul is a full-width 512-row stream (no 128-row stubs).
    out = nc.dram_tensor("out", [NHID_LOC, S], BF16, kind="ExternalOutput")

    xT_r = xT.rearrange("(t p) w -> p t w", p=128)        # [128, 20, 2048]
    wq_r = wq.rearrange("(t p) n -> p t n", p=128)        # [128, 20, 512]
    wk_r = wk.rearrange("(t p) n -> p t n", p=128)
    wv_r = wv.rearrange("(t p) n -> p t n", p=128)
    wo_r = wo.rearrange("(t p) n -> p t n", p=128)        # [128, 16, 640]
    cosT_r = cosT.rearrange("(d p) w -> p d w", p=128)    # [128, 2, 2048]
    rsinT_r = rsinT.rearrange("(d p) w -> p d w", p=128)
    qnw_r = qnw.rearrange("(d p) o -> p (d o)", p=128)    # [128, 2]
    knw_r = knw.rearrange("(d p) o -> p (d o)", p=128)

    with ExitStack() as ctx:
        tc = ctx.enter_context(tile.TileContext(nc))
        ec = ctx.enter_context
        constp = ec(tc.tile_pool(name="const", bufs=1))
        persist = ec(tc.tile_pool(name="persist", bufs=1))
        dram = ec(tc.tile_pool(name="dram", bufs=1, space="DRAM"))
        # PSUM: 8 banks total.  b: 5 rotating [128,512] matmul targets;
        # r: sum-of-(squares|exps) broadcast rows; acc: shared 2-bank slot
        # for the attention O accumulator and the o-proj accumulator.
        psB = ec(tc.tile_pool(name="psB", bufs=5, space="PSUM"))
        psR = ec(tc.tile_pool(name="psR", bufs=1, space="PSUM"))
        psA = ec(tc.tile_pool(name="psA", bufs=1, space="PSUM"))
        xtp = ec(tc.tile_pool(name="xtp", bufs=2))
        scr = ec(tc.tile_pool(name="scr", bufs=2))
        esb = ec(tc.tile_pool(name="esb", bufs=5))
        attp = ec(tc.tile_pool(name="attp", bufs=2))
        agp = ec(tc.tile_pool(name="agp", bufs=1))
        outp = ec(tc.tile_pool(name="outp", bufs=2))

        # ---- constants (live for the whole kernel) ----
        wq_s = constp.tile([128, KT, 512], BF16)
        wk_s = constp.tile([128, KT, 256], BF16)
        wv_s = constp.tile([128, KT, 256], BF16)
        wo_s = constp.tile([128, TM, NHID_LOC], BF16)
        cos_s = constp.tile([128, 2, S], BF16)
        rsin_s = constp.tile([128, 2, S], BF16)
        qn_s = constp.tile([128, 2], F32)
        kn_s = constp.tile([128, 2], F32)
        mask_s = constp.tile([128, 1152], BF16)
        onesm = constp.tile([128, 128], BF16)
        nc.vector.memset(onesm[:], 1.0)

        # persistent activations
        qT_s = persist.tile([128, 2, 2, S], BF16)   # [p, head, hd-chunk, tok]
        kT_s = persist.tile([128, 2, S], BF16)      # [p, hd-chunk, tok]
        v_s = persist.tile([128, TM, 256], BF16)    # [p(tok), tok-tile, hd]

        pending = []   # (issue_att_idx, qb, agout) collectives in flight
        staged = []    # (qb, ag_s) gathered results staged in SBUF
        att_count = [0]
        xt_tiles = {}

        def load_xt(tc4):
            t = xtp.tile([128, KT, QB], BF16, tag="xt")
            for g0 in range(0, KT, 5):
                nc.sync.dma_start(
                    t[:, g0:g0 + 5, :],
                    xT_r[:, g0:g0 + 5, tc4 * QB:(tc4 + 1) * QB],
                )
            xt_tiles[tc4] = t

        def rms_rope(ps_pair, nw_s, dst, dst_h, t0):
            """ps_pair: two [128, QB] f32 PSUM chunks of one head's ^T proj.
            Normalize (RMS over the 256 partition dims), scale by per-dim
            weight, apply RoPE, write bf16 into dst[:, (dst_h,) d, t0:t0+QB].
            All row-vectors are kept 128-partition-broadcast (via the ones
            matmul) so every DVE/ACT op runs full-width.
            """
            sqs = []
            for d in range(2):
                sq = scr.tile([128, QB], BF16, tag="sq%d" % d)
                nc.scalar.activation(sq[:], ps_pair[d][:], AF.Square)
                sqs.append(sq)
            nc.vector.tensor_add(sqs[0][:], sqs[0][:], sqs[1][:])
            ss = psR.tile([128, QB], F32, tag="r")
            nc.tensor.matmul(ss[:], onesm[:], sqs[0][:], start=True, stop=True)
            sr = scr.tile([128, QB], F32, tag="sr")
            nc.scalar.activation(sr[:], ss[:], AF.Sqrt, scale=1.0 / HD)
            rr = scr.tile([128, QB], F32, tag="rr")
            nc.vector.reciprocal_approx_fast(rr[:], sr[:])
            tqs = []
            for d in range(2):
                tq = scr.tile([128, QB], BF16, tag="tq%d" % d)
                nc.vector.scalar_tensor_tensor(
                    tq[:], ps_pair[d][:], nw_s[:, d:d + 1], rr[:],
                    ALU.mult, ALU.mult,
                )
                tqs.append(tq)
            for d in range(2):
                a = scr.tile([128, QB], BF16, tag="ra")
                b = scr.tile([128, QB], BF16, tag="rb")
                nc.vector.tensor_mul(a[:], tqs[d][:], cos_s[:, d, t0:t0 + QB])
                nc.vector.tensor_mul(b[:], tqs[1 - d][:], rsin_s[:, d, t0:t0 + QB])
                if dst_h is None:
                    dslice = dst[:, d, t0:t0 + QB]
                else:
                    dslice = dst[:, dst_h, d, t0:t0 + QB]
                nc.vector.tensor_add(dslice, a[:], b[:])

        def proj_chunk(tc4, fills=()):
            """Emit one 512-token projection chunk.  `fills` is a list of
            thunks (o-proj m-tiles) interleaved between the matmul chains so
            their PSUM-drain WARs overlap long independent PE work."""
            fills = list(fills)
            xs = xt_tiles.pop(tc4)[:, :, :]
            t0 = tc4 * QB
            # K^T for this token chunk
            kps = []
            for d in range(2):
                pk = psB.tile([128, QB], F32, tag="b")
                for kt in range(KT):
                    nc.tensor.matmul(
                        pk[:], wk_s[:, kt, d * 128:(d + 1) * 128],
                        xs[:, kt, :], start=(kt == 0), stop=(kt == KT - 1),
                    )
                kps.append(pk)
            rms_rope(kps, kn_s, kT_s, None, t0)
            if fills:
                fills.pop(0)()
            # Q^T per head
            for h in range(2):
                qps = []
                for d in range(2):
                    pq = psB.tile([128, QB], F32, tag="b")
                    c = 2 * h + d
                    for kt in range(KT):
                        nc.tensor.matmul(
                            pq[:], wq_s[:, kt, c * 128:(c + 1) * 128],
                            xs[:, kt, :], start=(kt == 0), stop=(kt == KT - 1),
                        )
                    qps.append(pq)
                rms_rope(qps, qn_s, qT_s, h, t0)
                if fills:
                    fills.pop(0)()
            # V (natural layout) for the 4 token tiles in this chunk
            for mm in range(4):
                m = (t0 // 128) + mm
                pv = psB.tile([128, QB], F32, tag="b")
                for kt in range(KT):
                    nc.tensor.matmul(
                        pv[:, 0:256], xs[:, kt, mm * 128:(mm + 1) * 128],
                        wv_s[:, kt, :], start=(kt == 0), stop=(kt == KT - 1),
                    )
                nc.vector.tensor_copy(v_s[:, m, :], pv[:, 0:256])
                if fills and mm in (0, 2):
                    fills.pop(0)()
            for f in fills:
                f()

        def att_qb(rep, qb):
            q0 = qb * QB
            kt_lo = max(0, (q0 - WIN) // 128)
            kt_hi = (q0 + QB - 1) // 128
            att = attp.tile([128, 4, QB], BF16, tag="att")
            for h in range(2):
                o_ps = psA.tile([128, 2, QB], F32, tag="acc")
                # softmax denominator: exp tiles are summed on DVE as they
                # are produced (esum), then ONE ones-matmul reduces over the
                # key partitions — instead of one PE matmul per k-tile.
                esum = attp.tile([128, QB], BF16, tag="esum")
                nc.vector.memset(esum[:], 0.0)
                kts = list(range(kt_lo, kt_hi + 1))
                # valid q-column range per k-tile: edge tiles are narrowed
                # (outside columns are fully masked -> skip their compute)
                rng = {}
                for kt in kts:
                    d_off = q0 - kt * 128
                    lo_q = max(0, -d_off)
                    hi_q = min(QB, WIN + 128 - d_off) if d_off >= 640 else QB
                    rng[kt] = (lo_q, hi_q)
                # PV accumulation order: a full-width tile first (so every
                # psum column's first write has start=True), then the rest.
                full0 = next(kt for kt in kts if rng[kt] == (0, QB))
                pv_order = [full0] + [kt for kt in kts if kt != full0]
                epipe = {}
                emitted = [0]

                def emit_pv(kt):
                    first, last = kt == pv_order[0], kt == pv_order[-1]
                    lo_q, hi_q = rng[kt]
                    e = epipe.pop(kt)
                    for d in range(2):
                        nc.tensor.matmul(
                            o_ps[:, d, lo_q:hi_q],
                            v_s[:, kt, d * 128:(d + 1) * 128],
                            e[:, lo_q:hi_q], start=first, stop=last,
                        )

                def drain_pv(upto):
                    # emit pending PVs in pv_order whose exp tile exists
                    # and whose scores stage is >= 2 iterations old
                    while emitted[0] < len(pv_order):
                        kt = pv_order[emitted[0]]
                        if kt not in epipe or kts.index(kt) > upto:
                            break
                        emit_pv(kt)
                        emitted[0] += 1

                for i, kt in enumerate(kts):
                    k0 = kt * 128
                    d_off = q0 - k0
                    lo_q, hi_q = rng[kt]
                    s_ps = psB.tile([128, QB], F32, tag="b")
                    for d in range(2):
                        nc.tensor.matmul(
                            s_ps[:, lo_q:hi_q], kT_s[:, d, k0:k0 + 128],
                            qT_s[:, h, d, q0 + lo_q:q0 + hi_q],
                            start=(d == 0), stop=(d == 1),
                        )
                    e_s = esb.tile([128, QB], BF16, tag="e")
                    nc.scalar.activation(e_s[:, lo_q:hi_q], s_ps[:, lo_q:hi_q],
                                         AF.Exp)
                    if not (128 <= d_off <= 512):
                        nc.vector.tensor_mul(
                            e_s[:, lo_q:hi_q], e_s[:, lo_q:hi_q],
                            mask_s[:, d_off + lo_q:d_off + hi_q],
                        )
                    nc.vector.tensor_add(esum[:, lo_q:hi_q],
                                         esum[:, lo_q:hi_q],
                                         e_s[:, lo_q:hi_q])
                    epipe[kt] = e_s
                    drain_pv(i - 2)
                drain_pv(len(kts) - 1)
                se_ps = psR.tile([128, QB], F32, tag="r")
                nc.tensor.matmul(se_ps[:], onesm[:], esum[:],
                                 start=True, stop=True)
                rc = scr.tile([128, QB], F32, tag="rc")
                nc.vector.reciprocal_approx_fast(rc[:], se_ps[:])
                for d in range(2):
                    nc.vector.tensor_mul(att[:, 2 * h + d, :], o_ps[:, d, :],
                                         rc[:])
            # AllGather this block's attention outputs across the batch
            # group; consumed by o-proj one phase later.
            agin = dram.tile([512, QB], BF16, tag=f"agin{rep}_{qb}")
            agout = dram.tile([NH * HD, QB], BF16, tag=f"agout{rep}_{qb}")
            agin_r = agin.rearrange("(c p) w -> c p w", p=128)
            for c in range(4):
                nc.sync.dma_start(agin_r[c], att[:, c, :])
            if stub_collective:
                agout_r = agout.rearrange("(r c p) w -> r c p w", p=128, c=4)
                for rr_i in range(4):
                    for cc in range(4):
                        nc.sync.dma_start(agout_r[rr_i, cc], att[:, cc, :])
            else:
                nc.gpsimd.collective_compute(
                    "AllGather",
                    mybir.AluOpType.bypass,
                    replica_groups=REPLICA_GROUPS,
                    ins=[agin[:]],
                    outs=[agout[:]],
                )
            pending.append((att_count[0], qb, agout))

        def stage_one():
            _, qb, agout = pending.pop(0)
            ag_s = agp.tile([128, TM, QB], BF16, tag="ag")
            agout_r2 = agout.rearrange("(t p) w -> p t w", p=128)
            # split along kt2 (not columns): the o-proj chains consume
            # ag_s[:, kt2, :] row-slices in order, so the first chain only
            # waits on the first quarter; rows are 1KB-contiguous in DRAM.
            for g in range(4):
                nc.sync.dma_start(
                    ag_s[:, 4 * g:4 * g + 4, :],
                    agout_r2[:, 4 * g:4 * g + 4, :],
                )
            staged.append((qb, ag_s))

        def maybe_stage():
            # Stage a gathered block only once its collective has had at
            # least one full attention phase (plus the interleaved phases)
            # of runway.
            if pending and not staged and pending[0][0] <= att_count[0] - 2:
                stage_one()

        def oproj_ochunk(qb, ag_s, j):
            # one 128-column chunk of the o-proj for this query block:
            # out^T[j*128:(j+1)*128, q0:q0+QB] — a single 16-step accumulation
            # chain of uniform 512-row matmuls.
            q0 = qb * QB
            po = psB.tile([128, QB], F32, tag="b")
            for kt2 in range(TM):
                nc.tensor.matmul(
                    po[:], wo_s[:, kt2, j * 128:(j + 1) * 128],
                    ag_s[:, kt2, :], start=(kt2 == 0), stop=(kt2 == TM - 1),
                )
            ot = outp.tile([128, QB], BF16, tag="ot")
            nc.vector.tensor_copy(ot[:], po[:])
            nc.sync.dma_start(out[j * 128:(j + 1) * 128, q0:q0 + QB], ot[:])

        def make_fills():
            if not staged:
                return []
            qb, ag_s = staged.pop(0)
            return [lambda j=j: oproj_ochunk(qb, ag_s, j)
                    for j in range(NHID_LOC // 128)]

        # Phase order per rep: attention phases are interleaved between
        # projection phases (legal: att qb0/qb1 need only token chunks 0-1,
        # qb2 needs chunk 2, qb3 needs chunk 3), which spreads the four
        # AllGathers ~evenly across the rep so they never queue on the CC
        # stream, and gives each one 2+ phases of runway before its o-proj
        # (the last three blocks drain during the NEXT rep's projections).
        for rep in range(reps):
            if rep == 0:
                t0c = xtp.tile([128, KT, QB], BF16, tag="xt")
                for a, b in ((0, 2), (2, 5), (5, 10), (10, 15), (15, 20)):
                    nc.sync.dma_start(t0c[:, a:b, :], xT_r[:, a:b, 0:QB])
                    nc.sync.dma_start(wq_s[:, a:b, :], wq_r[:, a:b, :])
                    nc.sync.dma_start(wk_s[:, a:b, :], wk_r[:, a:b, :])
                    nc.sync.dma_start(wv_s[:, a:b, :], wv_r[:, a:b, :])
                xt_tiles[0] = t0c
                nc.sync.dma_start(qn_s[:], qnw_r[:])
                nc.sync.dma_start(kn_s[:], knw_r[:])
                nc.sync.dma_start(cos_s[:], cosT_r[:])
                nc.sync.dma_start(rsin_s[:], rsinT_r[:])
                nc.sync.dma_start(mask_s[:], maskb[:])
                load_xt(1)
                nc.sync.dma_start(wo_s[:], wo_r[:])
            maybe_stage()
            proj_chunk(0, make_fills())
            maybe_stage()
            proj_chunk(1, make_fills())
            att_qb(rep, 0)
            att_count[0] += 1
            load_xt(2)
            maybe_stage()
            proj_chunk(2, make_fills())
            att_qb(rep, 1)
            att_count[0] += 1
            load_xt(3)
            maybe_stage()
            proj_chunk(3, make_fills())
            att_qb(rep, 2)
            att_count[0] += 1
            if rep + 1 < reps:
                load_xt(0)
            att_qb(rep, 3)
            att_count[0] += 1
            if rep + 1 < reps:
                load_xt(1)
        while pending or staged:
            if not staged:
                stage_one()
            for f in make_fills():
                f()

    nc.compile()
    return nc


_NC = {}


def _build(reps=1):
    if reps not in _NC:
        _NC[reps] = _emit(
            bacc.Bacc("TRN2", target_bir_lowering=False, debug=False,
                      num_devices=NCORES),
            reps=reps,
        )
    return _NC[reps]


def _host_prep(hidden_states, cos, sin, wq, wk, wv, wo, q_norm_w, k_norm_w):
    """Build the 8 per-core input maps (numpy, bf16 where device expects bf16)."""
    f32 = np.float32
    qn = ((1.0 + q_norm_w.astype(f32)) * SCALE).reshape(HD, 1)
    kn = (1.0 + k_norm_w.astype(f32)).reshape(HD, 1)
    # rsin: [-sin_firsthalf, +sin_secondhalf] so rope = q*cos + q[swap]*rsin
    # mask band: maskb[kk, y] = 1 iff 0 <= y - kk < WIN, where y = d_off + q
    # (q-position minus k-tile base; always >= 0 at the accessed offsets)
    kk = np.arange(128)[:, None]
    y = np.arange(1152)[None, :]
    maskb = ((y - kk >= 0) & (y - kk < WIN)).astype(NPBF16)

    in_maps = []
    for c in range(NCORES):
        b, g = divmod(c, 4)
        sin_b = sin[b].astype(f32)
        rsin = np.concatenate([-sin_b[:, :128], sin_b[:, 128:]], axis=1)
        in_maps.append({
            "xT": np.ascontiguousarray(hidden_states[b].T).astype(NPBF16),
            "wq": np.ascontiguousarray(
                wq[:, 2 * g * HD:(2 * g + 2) * HD]).astype(NPBF16),
            "wk": np.ascontiguousarray(wk[:, g * HD:(g + 1) * HD]).astype(NPBF16),
            "wv": np.ascontiguousarray(wv[:, g * HD:(g + 1) * HD]).astype(NPBF16),
            "wo": np.ascontiguousarray(
                wo[:, g * NHID_LOC:(g + 1) * NHID_LOC]).astype(NPBF16),
            "cosT": np.ascontiguousarray(cos[b].T).astype(NPBF16),
            "rsinT": np.ascontiguousarray(rsin.T).astype(NPBF16),
            "qnw": qn,
            "knw": kn,
            "maskb": maskb,
        })
    return in_maps


class _Runner:
    """Compile the Bass module to a reusable 8-device PJRT executable
    (mirrors bass2jax.run_bass_via_pjrt but keeps the jitted fn for
    repeated steady-state invocation)."""

    def __init__(self, nc):
        import jax
        from jax.sharding import Mesh, PartitionSpec
        try:
            from jax import shard_map as _sm
            shard_map = _sm.shard_map if hasattr(_sm, "shard_map") else _sm
        except Exception:
            from jax.experimental.shard_map import shard_map
        from concourse import bass2jax
        from concourse.bass2jax import _bass_exec_p

        bass2jax.install_neuronx_cc_hook()
        self.jax = jax
        self.nc = nc
        part_name = (nc.partition_id_tensor.name
                     if nc.partition_id_tensor else None)
        in_names, out_names, out_avals = [], [], []
        for alloc in nc.m.functions[0].allocations:
            if not isinstance(alloc, mybir.MemoryLocationSet):
                continue
            name = alloc.memorylocations[0].name
            if alloc.kind == "ExternalInput":
                if name != part_name:
                    in_names.append(name)
            elif alloc.kind == "ExternalOutput":
                out_names.append(name)
                out_avals.append(jax.core.ShapedArray(
                    tuple(alloc.tensor_shape), mybir.dt.np(alloc.dtype)))
        self.in_names, self.out_names, self.out_avals = in_names, out_names, out_avals
        all_names = list(in_names) + list(out_names)
        if part_name is not None:
            all_names.append(part_name)

        def _body(*args):
            operands = list(args)
            if part_name is not None:
                operands.append(bass2jax.partition_id_tensor())
            outs = _bass_exec_p.bind(
                *operands,
                out_avals=tuple(out_avals),
                in_names=tuple(all_names),
                out_names=tuple(out_names),
                lowering_input_output_aliases=(),
                sim_require_finite=True,
                sim_require_nnan=True,
                nc=nc,
            )
            return tuple(outs)

        devices = jax.devices()[:NCORES]
        self.mesh = Mesh(np.asarray(devices), ("core",))
        n_args = len(in_names) + len(out_names)
        self.fn = jax.jit(
            shard_map(
                _body, mesh=self.mesh,
                in_specs=(PartitionSpec("core"),) * n_args,
                out_specs=(PartitionSpec("core"),) * len(out_names),
                check_vma=False,
            ),
            keep_unused=True,
        )
        self.sharding = jax.sharding.NamedSharding(
            self.mesh, PartitionSpec("core"))
        self.zeros = [
            jax.device_put(
                np.zeros((NCORES * a.shape[0], *a.shape[1:]), a.dtype),
                self.sharding)
            for a in out_avals
        ]

    def put(self, in_maps):
        concat = [
            np.concatenate([np.asarray(in_maps[c][n]) for c in range(NCORES)],
                           axis=0)
            for n in self.in_names
        ]
        return [self.jax.device_put(a, self.sharding) for a in concat]

    def run(self, in_dev):
        outs = self.fn(*in_dev, *self.zeros)
        return [o.block_until_ready() for o in outs]

    def results(self, outs):
        per_core = []
        for c in range(NCORES):
            m = {}
            for i, n in enumerate(self.out_names):
                a = self.out_avals[i]
                m[n] = np.asarray(outs[i]).reshape(NCORES, *a.shape)[c]
            per_core.append(m)
        return per_core


_RUNNER = None


def _get_runner():
    global _RUNNER
    if _RUNNER is None:
        _RUNNER = _Runner(_build())
    return _RUNNER


def kernel(hidden_states, cos, sin, wq, wk, wv, wo, q_norm_w, k_norm_w):
    global _RUNNER
    in_maps = _host_prep(hidden_states, cos, sin, wq, wk, wv, wo,
                         q_norm_w, k_norm_w)
    # The axon tunnel fails transiently (~1/3 of runs: mesh desync / exec-unit
    # unrecoverable). Retry the dispatch; on repeat failure rebuild the runner.
    last = None
    for attempt in range(4):
        try:
            r = _get_runner()
            res = r.results(r.run(r.put(in_maps)))
            break
        except Exception as e:  # transient axon/NRT dispatch failures
            last = e
            _RUNNER = None
    else:
        raise last
    out = np.empty((B, S, HID), np.float32)
    for b in range(B):
        out[b] = np.concatenate(
            [res[b * 4 + g]["out"].T.astype(np.float32) for g in range(4)],
            axis=1,
        )
    return out
